# revision 43
# baseline (speedup 1.0000x reference)
"""Trainium2 Bass kernel for a fused attention block (B=4, C=256, N=2048, H=8).

Sharding: 8 cores = 4 batches x 2 head-groups (4 heads each). Host sums the
two head-group partial projections per batch.

Per-core pipeline. ScalarE exp is the bottleneck engine (128 ACTIVATEs of
FD=1024 ~= 132us); everything else is scheduled to hide underneath it:
  - S^T = K^T Q per m-tile (128 keys) x chunk (512 queries): all four heads'
    matmuls back-to-back as 4 concurrent PE row-tiles; two [128,1024] "st"
    PSUM tiles per m-tile rotate through 3 buffers so the PE can run a slot
    ahead of the exp stream.
  - AV with a ones-column appended to V^T (the softmax denominator falls out
    of the same accumulation) using PE *column* tiling: two concurrent
    33-wide matmuls at col positions 0/64, one PSUM bank per head pair.
    AV drains lag the exps (FIRST_AV_SLOT) and leftovers carry into the next
    chunk's first slot so the exp feed never parks behind them.
  - normalization (deferred one chunk): denominator rows are copied compact,
    inverted on ScalarE via 1/x = exp(-ln x) (Ln/Exp share one ACT table
    set, and this keeps the DVE reciprocal's drain off the critical path),
    bounced through DRAM and broadcast-read back 32-wide; one tensor_tensor
    mul per head pair. Garbage partition bands multiply by preset 1.0s and
    are killed by zero rows in the host-padded projection stationaries
    (spread layout: h_even rows 0-31, h_odd rows 64-95 - no partition moves).
  - output projection accumulates both head-pair tiles into one PSUM tile.
"""

import numpy as np

import bass_rust
import concourse.bass as bass
import concourse.mybir as mybir
import concourse.bass_utils as bass_utils
from concourse.tile import TileContext

B, C, N, H, HD = 4, 256, 2048, 8, 32
SCALE = float(HD) ** 0.5
NCORES = 8
HPC = H // 2            # heads per core (4)
NCHUNK = 512            # n (query) columns processed per chunk
NJ = N // NCHUNK        # 4
MTILES = N // 128       # 16 key/m tiles
F32 = mybir.dt.float32
MM_DT = mybir.dt.bfloat16

FIRST_AV_SLOT = 8       # m-tile slot where a chunk's first AV matmuls go
PRIO_OFFSET = 0         # priority boost for the S^T/exp feed (0 = off)
# DVE-exp offload: these m-tiles' heads-01 exp runs on VectorE instead of
# ScalarE (key -> slot where the deferred AV is emitted)
OFFLOAD_MTS = {}
# e^u-1 ~= (((A*u + B)*u + C)*u + D)*u on u in [-0.85, 0.85]; e^s = (1+h)^8
EXP_A, EXP_B, EXP_C, EXP_D = (0.04279558354861696, 0.17190282193923595,
                              0.499824485231412, 0.9996197378604269)
DEBUG = False           # adds chunk-0 intermediate dumps as extra outputs
LABELS = {}             # instruction name -> semantic label (for trace analysis)


def _lbl(inst, label):
    try:
        LABELS[inst.ins.name] = label
    except Exception:
        pass
    return inst


def _split_sync_waits(nc, max_waits=1):
    """This walrus build rejects instructions with >1 sync wait. Move extra
    waits onto preceding same-engine NoOps (engine stalls there instead)."""
    ctr = 0
    for f in nc.m.functions:
        for bb in f.blocks:
            out = []
            for inst in bb.instructions:
                si = inst.sync_info
                if si is not None and si.on_wait and len(si.on_wait) > max_waits:
                    waits = list(si.on_wait)
                    head, keep = waits[:-max_waits], waits[-max_waits:]
                    for i in range(0, len(head), max_waits):
                        nop = bass_rust.InstNoOp(name=f"wsplit-{ctr}")
                        ctr += 1
                        nop.engine = inst.engine
                        nop.sync_info = mybir.SyncInfo(
                            on_wait=head[i:i + max_waits], on_update=[]
                        )
                        nc.register_instruction(nop, overwrite=True)
                        out.append(nop)
                    inst.sync_info = mybir.SyncInfo(
                        on_wait=keep, on_update=list(si.on_update)
                    )
                out.append(inst)
            bb.instructions = out


def build_program():
    sdt = MM_DT
    nc = bass.Bass()

    # Host pre-chunks c (=256) into [128, 2, .] so partition dim is 128.
    x_in = nc.dram_tensor("x_in", [128, 2 * N], sdt, kind="ExternalInput")
    wq_in = nc.dram_tensor("wq_in", [128, 256], sdt, kind="ExternalInput")
    wk_in = nc.dram_tensor("wk_in", [128, 256], sdt, kind="ExternalInput")
    wv_in = nc.dram_tensor("wv_in", [128, 256], sdt, kind="ExternalInput")
    # wp_in: two zero-padded spread stationaries [wp1 | wp2], each [128, 256]
    wp_in = nc.dram_tensor("wp_in", [128, 512], sdt, kind="ExternalInput")
    y_out = nc.dram_tensor("y_out", [256, N], F32, kind="ExternalOutput")
    dbg = {}
    if DEBUG:
        for nm, shp, dt in [
                ("dbg_den", [128, NCHUNK], F32), ("dbg_r", [128, NCHUNK], F32),
                ("dbg_bc", [128, 1024], F32), ("dbg_attn", [128, 1024], MM_DT),
                ("dbg_ex", [128, 2048], MM_DT), ("dbg_q", [128, N], MM_DT),
                ("dbg_k", [128, N], MM_DT), ("dbg_v", [128, HPC * MTILES * 33], MM_DT)]:
            dbg[nm] = nc.dram_tensor(nm, shp, dt, kind="ExternalOutput")

    with TileContext(nc) as tc:
        with (
            tc.tile_pool(name="persist", bufs=1) as pp,
            tc.tile_pool(name="exps", bufs=8) as xp,
            tc.tile_pool(name="work", bufs=2) as wk_pool,
            tc.tile_pool(name="psum", bufs=1, space="PSUM") as stp,
            tc.tile_pool(name="drp", bufs=1, space="DRAM") as drp,
        ):
            # ---- preamble --------------------------------------------------
            # Tiny exp first thing: pulls the ~2.7us ACT table load under the
            # input DMAs instead of paying it before the first real exp.
            warm = pp.tile([1, 8], F32)
            nc.vector.memset(warm[:, :], 0.0)
            eng = nc.scalar
            eng.add_instruction(mybir.InstActivation(
                name=nc.get_next_instruction_name(),
                func=mybir.ActivationFunctionType.Exp,
                ins=[eng.lower_ap(warm[:, :]),
                     mybir.ImmediateValue(dtype=F32, value=0.0),
                     mybir.ImmediateValue(dtype=F32, value=1.0),
                     mybir.ImmediateValue(dtype=F32, value=0.0)],
                outs=[eng.lower_ap(warm[:, :])],
            ))

            # input DMAs; x ordered so K-half0/Q-chunk01 operands land first
            x_mm = pp.tile([128, 2 * N], sdt)
            w_mm = pp.tile([128, 3 * 256 + 512], sdt)
            for i, dsrc in enumerate((wq_in, wk_in, wv_in)):
                nc.gpsimd.dma_start(w_mm[:, i * 256:(i + 1) * 256], dsrc[:, :])
            nc.gpsimd.dma_start(w_mm[:, 768:1280], wp_in[:, :])
            nc.sync.dma_start(x_mm[:, 0:512], x_in[:, 0:512])
            nc.sync.dma_start(x_mm[:, N:N + 512], x_in[:, N:N + 512])
            nc.sync.dma_start(x_mm[:, 512:1024], x_in[:, 512:1024])
            nc.sync.dma_start(x_mm[:, N + 512:N + 1024], x_in[:, N + 512:N + 1024])
            nc.sync.dma_start(x_mm[:, 1024:N], x_in[:, 1024:N])
            nc.sync.dma_start(x_mm[:, N + 1024:2 * N], x_in[:, N + 1024:2 * N])
            wq_sb = w_mm[:, 0:256]
            wk_sb = w_mm[:, 256:512]
            wv_sb = w_mm[:, 512:768]
            wp_sb = w_mm[:, 768:1280]   # [wp1 | wp2] spread stationaries

            q_sb = pp.tile([128, N], sdt)
            k_sb = pp.tile([128, N], sdt)

            # V^T with ones column: per (head, mtile) a [128, 33] block.
            ones_f32 = pp.tile([128, 1], F32)
            nc.vector.memset(ones_f32[:, :], 1.0)
            vaug = pp.tile([128, HPC * MTILES * 33], sdt)
            nc.vector.tensor_copy(
                vaug.rearrange("p (b c) -> p b c", c=33)[:, :, 32:33],
                ones_f32[:, 0:1].to_broadcast([128, HPC * MTILES, 1]),
            )
            vaug_v = vaug.rearrange("p (h t c) -> p h t c", h=HPC, t=MTILES)

            # ones stationary for the K=1 recip-broadcast matmuls
            ones32 = pp.tile([128, 32], sdt)
            nc.vector.memset(ones32[:, :], 1.0)

            # AV accumulators keep partition bands 33-63 / 97-127 unwritten;
            # zero them once so the normalize mul stays finite forever.
            acc_init = []
            for tag in ("acc_ab", "acc_cd"):
                t = stp.tile([128, NCHUNK], F32, tag=tag, name=f"init_{tag}")
                nc.vector.memset(t[:, :], 0.0)
                acc_init.append(t)

            # ---- projection emitters (into the aux PSUM tile) --------------
            def emit_qk_half(dst, wsb, half):
                qp = stp.tile([128, 1024], F32, tag="st", bufs=3, name="qp")
                for s in range(2):
                    col0 = half * 1024 + s * 512
                    for cc in range(2):
                        _lbl(nc.tensor.matmul(
                            qp[:, s * 512:(s + 1) * 512],
                            wsb[:, cc * 128:(cc + 1) * 128],
                            x_mm[:, cc * N + col0: cc * N + col0 + 512],
                            start=(cc == 0), stop=(cc == 1),
                        ), f"QKPROJ h{half} s{s}")
                nc.vector.tensor_copy(dst[:, half * 1024:half * 1024 + 512],
                                      qp[:, 0:512])
                nc.vector.tensor_copy(dst[:, half * 1024 + 512:(half + 1) * 1024],
                                      qp[:, 512:1024])

            vhalf_ps = {}

            def emit_v_quarter(q):
                # 4 m-tiles of V^T; two quarters share one aux tile
                g, sub = q // 2, q % 2
                if sub == 0:
                    vhalf_ps[g] = stp.tile([128, 1024], F32, tag="st", bufs=3, name="vp")
                vp = vhalf_ps[g]
                for mtl in range(4):
                    vmt = q * 4 + mtl
                    for cc in range(2):
                        _lbl(nc.tensor.matmul(
                            vp[:, (sub * 4 + mtl) * 128:(sub * 4 + mtl + 1) * 128],
                            x_mm[:, cc * N + vmt * 128: cc * N + (vmt + 1) * 128],
                            wv_sb[:, cc * 128:(cc + 1) * 128],
                            start=(cc == 0), stop=(cc == 1),
                        ), f"VPROJ q{q} mtl{mtl}")
                if sub == 1:
                    nc.vector.tensor_copy(
                        vaug_v[:, :, g * 8:(g + 1) * 8, 0:32],
                        vp.rearrange("p (t h d) -> p h t d", t=8, h=HPC),
                    )

            def emit_exp(out_ap, in_ap, label="EXP"):
                ins = [eng.lower_ap(in_ap),
                       mybir.ImmediateValue(dtype=F32, value=0.0),
                       mybir.ImmediateValue(dtype=F32, value=1.0 / SCALE),
                       mybir.ImmediateValue(dtype=F32, value=0.0)]
                _lbl(eng.add_instruction(mybir.InstActivation(
                    name=nc.get_next_instruction_name(),
                    func=mybir.ActivationFunctionType.Exp,
                    ins=ins, outs=[eng.lower_ap(out_ap)],
                )), label)

            from concourse.alu_op_type import AluOpType

            def emit_dve_exp(ex_slice, st_tile):
                u = wk_pool.tile([128, 1024], sdt, tag="dveu", name="u")
                nc.vector.tensor_scalar_mul(
                    u[:, :], st_tile[:, :], 1.0 / (SCALE * 8.0))
                pa = wk_pool.tile([128, 1024], sdt, tag="dvepa", name="pa")
                pb = wk_pool.tile([128, 1024], sdt, tag="dvepb", name="pb")
                nc.vector.tensor_scalar_mul(pa[:, :], u[:, :], EXP_A)
                cur, nxt = pa, pb
                for coef in (EXP_B, EXP_C, EXP_D):
                    nc.vector.scalar_tensor_tensor(
                        nxt[:, :], cur[:, :], coef, u[:, :],
                        AluOpType.add, AluOpType.mult)
                    cur, nxt = nxt, cur
                for _ in range(3):
                    nc.vector.scalar_tensor_tensor(
                        nxt[:, :], cur[:, :], 2.0, cur[:, :],
                        AluOpType.add, AluOpType.mult)
                    cur, nxt = nxt, cur
                nc.vector.tensor_scalar_add(ex_slice, cur[:, :], 1.0)

            # ---- deferred normalize + projection ---------------------------
            # state for chunk j, filled during chunk j, consumed in chunk j+1
            norm_state = {}

            def emit_extract(j, tail=False):
                st = norm_state[j]
                den_sb = wk_pool.tile([128, NCHUNK], F32, tag="den", name="den_sb")
                st["den_sb"] = den_sb
                for i, acc in enumerate(st["accs"]):
                    for s in range(2):
                        r = 32 * (2 * i + s)
                        # at the tail ScalarE is idle: split the extraction
                        if tail and i == 0:
                            nc.scalar.copy(
                                den_sb[r:r + 1, :],
                                acc[s * 64 + 32:s * 64 + 33, :],
                            )
                        else:
                            nc.vector.tensor_copy(
                                den_sb[r:r + 1, :],
                                acc[s * 64 + 32:s * 64 + 33, :],
                            )

            def emit_recip(j):
                # 1/x = exp(-ln x) on ScalarE: Ln and Exp share one ACT table
                # set, and doing this on ScalarE keeps the DVE (and its drain)
                # off the muls' critical path.
                st = norm_state[j]
                den_sb = st["den_sb"]
                r_sb = wk_pool.tile([128, NCHUNK], sdt, tag="rsb", name="r_sb")
                ln_t = wk_pool.tile([128, NCHUNK], F32, tag="lnt", name="ln_t")
                nc.scalar.activation(
                    ln_t[:, :], den_sb[:, :], mybir.ActivationFunctionType.Ln)
                nc.scalar.activation(
                    r_sb[:, :], ln_t[:, :], mybir.ActivationFunctionType.Exp,
                    scale=-1.0)
                st["r_sb"] = r_sb
                if DEBUG and j == 0:
                    nc.sync.dma_start(dbg["dbg_den"][:, :], den_sb[:, :])
                    nc.sync.dma_start(dbg["dbg_r"][:, :], r_sb[:, :])

            def emit_bcast(j, tail=False):
                st = norm_state[j]
                r_sb = st["r_sb"]
                ps = stp.tile([128, 1024], F32, tag="st", bufs=3, name="psbc")
                for i in range(2):
                    for sub in range(2):
                        row = 32 * (2 * i + sub)
                        nc.tensor.matmul(
                            ps[64 * sub:64 * sub + 32, i * 512:(i + 1) * 512],
                            ones32[row:row + 1, :],
                            r_sb[row:row + 1, :],
                            tile_position=(row, 64 * sub),
                        )
                bc = wk_pool.tile([128, 1024], F32, tag="bc", name="bc")
                st["bc"] = bc
                nc.vector.tensor_copy(bc[:, :], ps[:, :])
                if DEBUG and j == 0:
                    nc.sync.dma_start(dbg["dbg_bc"][:, :], bc[:, :])

            def emit_norm(j):
                st = norm_state[j]
                bc = st["bc"]
                attn = wk_pool.tile([128, 1024], sdt, tag="attn", name="attn")
                st["attn"] = attn
                for i, acc in enumerate(st["accs"]):
                    nc.vector.tensor_mul(
                        attn[:, i * 512:(i + 1) * 512], acc[:, :],
                        bc[:, i * 512:(i + 1) * 512],
                    )
                if DEBUG and j == 0:
                    nc.sync.dma_start(dbg["dbg_attn"][:, :], attn[:, :])

            def emit_proj(j, tail=False):
                st = norm_state.pop(j)
                attn = st["attn"]
                n0 = j * NCHUNK
                yp = stp.tile([128, 1024], F32, tag="st", bufs=3, name="yp")
                for oh in range(2):
                    for i in range(2):
                        _lbl(nc.tensor.matmul(
                            yp[:, oh * 512:oh * 512 + NCHUNK],
                            wp_sb[:, i * 256 + oh * 128: i * 256 + (oh + 1) * 128],
                            attn[:, i * 512:(i + 1) * 512],
                            start=(i == 0), stop=(i == 1),
                        ), f"PROJ j{j} oh{oh} i{i}")
                for oh in range(2):
                    y_sb = wk_pool.tile([128, NCHUNK], F32, tag="ysb", name="y_sb")
                    if tail and oh == 0:
                        nc.scalar.copy(y_sb[:, :], yp[:, oh * 512:(oh + 1) * 512])
                    else:
                        nc.vector.tensor_copy(y_sb[:, :], yp[:, oh * 512:(oh + 1) * 512])
                    nc.sync.dma_start(
                        y_out[oh * 128:(oh + 1) * 128, n0:n0 + NCHUNK],
                        y_sb[:, :],
                    )

            # ---- up-front projections (rest interleave into chunk 0) -------
            # K-half0 / Q-chunk01 interleaved at sub-512 granularity so the
            # first S^T (needs k[:, 0:128] + q[:, 0:512]) fires after only
            # 4 matmuls + 2 copies instead of the full 8+4 chain.
            kp0 = stp.tile([128, 1024], F32, tag="st", bufs=3, name="kp0")
            qp0 = stp.tile([128, 1024], F32, tag="st", bufs=3, name="qp0")
            for sub in range(2):
                for dst_ps, wsb in ((kp0, wk_sb), (qp0, wq_sb)):
                    for cc in range(2):
                        _lbl(nc.tensor.matmul(
                            dst_ps[:, sub * 512:(sub + 1) * 512],
                            wsb[:, cc * 128:(cc + 1) * 128],
                            x_mm[:, cc * N + sub * 512: cc * N + (sub + 1) * 512],
                            start=(cc == 0), stop=(cc == 1),
                        ), f"QK0 s{sub}")
                # k-copy on the (still idle) ScalarE, q-copy on DVE: the
                # two run in parallel so the first S^T starts sooner
                nc.scalar.copy(
                    k_sb[:, sub * 512:(sub + 1) * 512],
                    kp0[:, sub * 512:(sub + 1) * 512])
                nc.vector.tensor_copy(
                    q_sb[:, sub * 512:(sub + 1) * 512],
                    qp0[:, sub * 512:(sub + 1) * 512])

            # per-slot extra work: (j, mt) -> list of callables
            slot_work = {}
            for j in range(NJ):
                if j == 0:
                    slot_work[(0, 0)] = [lambda: emit_v_quarter(0)]
                    slot_work[(0, 1)] = [lambda: emit_v_quarter(1)]
                    slot_work[(0, 2)] = [lambda: emit_qk_half(k_sb, wk_sb, 1)]
                    slot_work[(0, 3)] = [lambda: emit_v_quarter(2)]
                    slot_work[(0, 4)] = [lambda: emit_v_quarter(3)]
                    slot_work[(0, 8)] = [lambda: emit_qk_half(q_sb, wq_sb, 1)]
                else:
                    jm = j - 1
                    slot_work[(j, 1)] = [lambda jm=jm: emit_extract(jm)]
                    slot_work[(j, 3)] = [lambda jm=jm: emit_recip(jm)]
                    slot_work[(j, 4)] = [lambda jm=jm: emit_bcast(jm)]
                    slot_work[(j, 5)] = [lambda jm=jm: emit_norm(jm)]
                    slot_work[(j, 6)] = [lambda jm=jm: emit_proj(jm)]

            # ---- main attention loop ---------------------------------------
            # leftover AV closures carried into the next chunk's first slots
            carry_av = []
            for j in range(NJ):
                n0 = j * NCHUNK
                acc_ab = stp.tile([128, NCHUNK], F32, tag="acc_ab", name="acc_ab")
                acc_cd = stp.tile([128, NCHUNK], F32, tag="acc_cd", name="acc_cd")
                accs = [acc_ab, acc_cd]
                norm_state[j] = {"accs": accs}
                ex_tiles = {}
                pending_av = []
                deferred_av = {}

                def emit_av(mt, accs=accs, ex_tiles=ex_tiles):
                    ex_mt = ex_tiles.pop(mt)
                    for pair in range(2):
                        acc = accs[pair]
                        for sub in range(2):
                            h = 2 * pair + sub
                            _lbl(nc.tensor.matmul(
                                acc[sub * 64:sub * 64 + 33, :],
                                vaug_v[:, h, mt, :],
                                ex_mt[:, h * 512:(h + 1) * 512],
                                start=(mt == 0), stop=(mt == MTILES - 1),
                                tile_position=(0, sub * 64),
                            ), f"AV mt{mt} h{h}")

                for mt in range(MTILES):
                    # S^T first: all four heads' matmuls go back-to-back so
                    # the PE runs them as 4 concurrent row-tiles. Emitted at
                    # boosted priority so the scheduler never parks the exp
                    # feed behind AV / normalize work.
                    ex = xp.tile([128, 2048], sdt, name="ex")
                    ex_tiles[mt] = ex
                    with tc.high_priority(offset=PRIO_OFFSET):
                        st_1 = stp.tile([128, 1024], F32, tag="st", bufs=3, name="st_1")
                        st_2 = stp.tile([128, 1024], F32, tag="st", bufs=3, name="st_2")
                        for h in range(4):
                            dst = st_1 if h < 2 else st_2
                            _lbl(nc.tensor.matmul(
                                dst[:, (h % 2) * 512:(h % 2 + 1) * 512],
                                k_sb[h * 32:(h + 1) * 32, mt * 128:(mt + 1) * 128],
                                q_sb[h * 32:(h + 1) * 32, n0:n0 + NCHUNK],
                                tile_position=(32 * h, 0),
                            ), f"ST j{j} mt{mt} h{h}")
                        if mt in OFFLOAD_MTS:
                            emit_dve_exp(ex[:, 0:1024], st_1)
                        else:
                            emit_exp(ex[:, 0:1024], st_1[:, :], f"EXP j{j} mt{mt} a")
                        emit_exp(ex[:, 1024:2048], st_2[:, :], f"EXP j{j} mt{mt} b")
                    if mt in OFFLOAD_MTS:
                        deferred_av[OFFLOAD_MTS[mt]] = mt
                    else:
                        pending_av.append(mt)

                    # previous chunk's leftover AV matmuls, one per slot
                    if mt < 2 and carry_av:
                        carry_av.pop(0)()
                    if mt in deferred_av:
                        emit_av(deferred_av.pop(mt))
                    # AV drain: up to 2 one-slot-lagged m-tiles per slot
                    if mt >= FIRST_AV_SLOT:
                        for _ in range(2):
                            if pending_av and pending_av[0] <= mt - 1:
                                emit_av(pending_av.pop(0))

                    if DEBUG and j == 0 and mt == 0:
                        nc.sync.dma_start(dbg["dbg_ex"][:, :], ex[:, :])
                    if DEBUG and j == 0 and mt == 15:
                        nc.sync.dma_start(dbg["dbg_q"][:, :], q_sb[:, :])
                        nc.sync.dma_start(dbg["dbg_k"][:, :], k_sb[:, :])
                        nc.sync.dma_start(dbg["dbg_v"][:, :], vaug[:, :])

                    for fn in slot_work.get((j, mt), ()):
                        fn()

                if j < NJ - 1:
                    for mt_left in pending_av:
                        carry_av.append(lambda m=mt_left, e=emit_av: e(m))
                else:
                    while pending_av:
                        emit_av(pending_av.pop(0))

            # tail: last chunk's normalize + projection
            emit_extract(NJ - 1, tail=True)
            emit_recip(NJ - 1)
            emit_bcast(NJ - 1, tail=True)
            emit_norm(NJ - 1)
            emit_proj(NJ - 1, tail=True)

    _split_sync_waits(nc)
    return nc


_CACHE = {}


def _get_program():
    if "nc" not in _CACHE:
        _CACHE["nc"] = build_program()
    return _CACHE["nc"]


def _core_inputs(x, w_qkv, w_proj, core):
    b, g = core // 2, core % 2
    r0 = g * 128
    wq = w_qkv[r0:r0 + 128, :].T            # [256 c, 128 (h,d)]
    wk = w_qkv[256 + r0:256 + r0 + 128, :].T
    wv = w_qkv[512 + r0:512 + r0 + 128, :].T
    wpj = w_proj[:, r0:r0 + 128].T          # [128 c_local, 256 o]

    hdt = mybir.dt.np(MM_DT)

    def chunk_c(a):  # [256, m] -> [128, 2*m] with c split across 2 free-chunks
        m = a.shape[1]
        return np.ascontiguousarray(
            a.reshape(2, 128, m).transpose(1, 0, 2).reshape(128, 2 * m)
        ).astype(hdt)

    # spread + zero-pad the projection stationaries: head-pair i uses rows
    # {0-31: h_even dims, 64-95: h_odd dims}, other bands killed by zeros
    wp = np.zeros((128, 512), dtype=np.float32)
    for i in range(2):                      # head pair
        for s in range(2):                  # even/odd head in pair
            wp[s * 64:s * 64 + 32, i * 256:(i + 1) * 256] = \
                wpj[(2 * i + s) * 32:(2 * i + s + 1) * 32, :]

    return {
        "x_in": chunk_c(x[b]),
        "wq_in": chunk_c(wq),
        "wk_in": chunk_c(wk),
        "wv_in": chunk_c(wv),
        "wp_in": wp.astype(hdt),
    }


def kernel(x, w_qkv, w_proj, n_heads=8, _trace=False):
    x = np.asarray(x, dtype=np.float32)
    w_qkv = np.asarray(w_qkv, dtype=np.float32)
    w_proj = np.asarray(w_proj, dtype=np.float32)
    assert int(n_heads) == H

    nc = _get_program()
    in_maps = [_core_inputs(x, w_qkv, w_proj, core) for core in range(NCORES)]
    res = bass_utils.run_bass_kernel_spmd(
        nc, in_maps, core_ids=list(range(NCORES)), trace=_trace
    )
    parts = [res.results[core]["y_out"] for core in range(NCORES)]
    y = np.stack([parts[2 * b] + parts[2 * b + 1] for b in range(B)])
    if _trace:
        kernel.last_result = res
    return y.astype(np.float32)


# revision 45
# speedup vs baseline: 1.0203x; 1.0203x over previous
"""Trainium2 Bass kernel for a fused attention block (B=4, C=256, N=2048, H=8).

Sharding: 8 cores = 4 batches x 2 head-groups (4 heads each). Host sums the
two head-group partial projections per batch.

Per-core pipeline. ScalarE exp is the bottleneck engine (128 ACTIVATEs of
FD=1024 ~= 132us); everything else is scheduled to hide underneath it:
  - S^T = K^T Q per m-tile (128 keys) x chunk (512 queries): all four heads'
    matmuls back-to-back as 4 concurrent PE row-tiles; two [128,1024] "st"
    PSUM tiles per m-tile rotate through 3 buffers so the PE can run a slot
    ahead of the exp stream.
  - AV with a ones-column appended to V^T (the softmax denominator falls out
    of the same accumulation) using PE *column* tiling: two concurrent
    33-wide matmuls at col positions 0/64, one PSUM bank per head pair.
    AV drains lag the exps (FIRST_AV_SLOT) and leftovers carry into the next
    chunk's first slot so the exp feed never parks behind them.
  - normalization (deferred one chunk): denominator rows are copied compact,
    inverted on ScalarE via 1/x = exp(-ln x) (Ln/Exp share one ACT table
    set, and this keeps the DVE reciprocal's drain off the critical path),
    bounced through DRAM and broadcast-read back 32-wide; one tensor_tensor
    mul per head pair. Garbage partition bands multiply by preset 1.0s and
    are killed by zero rows in the host-padded projection stationaries
    (spread layout: h_even rows 0-31, h_odd rows 64-95 - no partition moves).
  - output projection accumulates both head-pair tiles into one PSUM tile.
"""

import numpy as np

import bass_rust
import concourse.bass as bass
import concourse.mybir as mybir
import concourse.bass_utils as bass_utils
from concourse.tile import TileContext

B, C, N, H, HD = 4, 256, 2048, 8, 32
SCALE = float(HD) ** 0.5
NCORES = 8
HPC = H // 2            # heads per core (4)
NCHUNK = 512            # n (query) columns processed per chunk
NJ = N // NCHUNK        # 4
MTILES = N // 128       # 16 key/m tiles
F32 = mybir.dt.float32
MM_DT = mybir.dt.bfloat16

FIRST_AV_SLOT = 8       # m-tile slot where a chunk's first AV matmuls go
PRIO_OFFSET = 0         # priority boost for the S^T/exp feed (0 = off)
# DVE-exp offload: these m-tiles' heads-01 exp runs on VectorE instead of
# ScalarE (key -> slot where the deferred AV is emitted)
OFFLOAD_MTS = {}
# e^u-1 ~= (((A*u + B)*u + C)*u + D)*u on u in [-0.85, 0.85]; e^s = (1+h)^8
EXP_A, EXP_B, EXP_C, EXP_D = (0.04279558354861696, 0.17190282193923595,
                              0.499824485231412, 0.9996197378604269)
DEBUG = False           # adds chunk-0 intermediate dumps as extra outputs
LABELS = {}             # instruction name -> semantic label (for trace analysis)


def _lbl(inst, label):
    try:
        LABELS[inst.ins.name] = label
    except Exception:
        pass
    return inst


def _split_sync_waits(nc, max_waits=1):
    """This walrus build rejects instructions with >1 sync wait. Move extra
    waits onto preceding same-engine NoOps (engine stalls there instead)."""
    ctr = 0
    for f in nc.m.functions:
        for bb in f.blocks:
            out = []
            for inst in bb.instructions:
                si = inst.sync_info
                if si is not None and si.on_wait and len(si.on_wait) > max_waits:
                    waits = list(si.on_wait)
                    head, keep = waits[:-max_waits], waits[-max_waits:]
                    for i in range(0, len(head), max_waits):
                        nop = bass_rust.InstNoOp(name=f"wsplit-{ctr}")
                        ctr += 1
                        nop.engine = inst.engine
                        nop.sync_info = mybir.SyncInfo(
                            on_wait=head[i:i + max_waits], on_update=[]
                        )
                        nc.register_instruction(nop, overwrite=True)
                        out.append(nop)
                    inst.sync_info = mybir.SyncInfo(
                        on_wait=keep, on_update=list(si.on_update)
                    )
                out.append(inst)
            bb.instructions = out


def build_program():
    sdt = MM_DT
    nc = bass.Bass()

    # Host pre-chunks c (=256) into [128, 2, .] so partition dim is 128.
    x_in = nc.dram_tensor("x_in", [128, 2 * N], sdt, kind="ExternalInput")
    wq_in = nc.dram_tensor("wq_in", [128, 256], sdt, kind="ExternalInput")
    wk_in = nc.dram_tensor("wk_in", [128, 256], sdt, kind="ExternalInput")
    wv_in = nc.dram_tensor("wv_in", [128, 256], sdt, kind="ExternalInput")
    # wp_in: two zero-padded spread stationaries [wp1 | wp2], each [128, 256]
    wp_in = nc.dram_tensor("wp_in", [128, 512], sdt, kind="ExternalInput")
    y_out = nc.dram_tensor("y_out", [256, N], MM_DT, kind="ExternalOutput")
    dbg = {}
    if DEBUG:
        for nm, shp, dt in [
                ("dbg_den", [128, NCHUNK], F32), ("dbg_r", [128, NCHUNK], F32),
                ("dbg_bc", [128, 1024], F32), ("dbg_attn", [128, 1024], MM_DT),
                ("dbg_ex", [128, 2048], MM_DT), ("dbg_q", [128, N], MM_DT),
                ("dbg_k", [128, N], MM_DT), ("dbg_v", [128, HPC * MTILES * 33], MM_DT)]:
            dbg[nm] = nc.dram_tensor(nm, shp, dt, kind="ExternalOutput")

    with TileContext(nc) as tc:
        with (
            tc.tile_pool(name="persist", bufs=1) as pp,
            tc.tile_pool(name="exps", bufs=8) as xp,
            tc.tile_pool(name="work", bufs=2) as wk_pool,
            tc.tile_pool(name="psum", bufs=1, space="PSUM") as stp,
            tc.tile_pool(name="drp", bufs=1, space="DRAM") as drp,
        ):
            # ---- preamble --------------------------------------------------
            # Tiny exp first thing: pulls the ~2.7us ACT table load under the
            # input DMAs instead of paying it before the first real exp.
            warm = pp.tile([1, 8], F32)
            nc.vector.memset(warm[:, :], 0.0)
            eng = nc.scalar
            eng.add_instruction(mybir.InstActivation(
                name=nc.get_next_instruction_name(),
                func=mybir.ActivationFunctionType.Exp,
                ins=[eng.lower_ap(warm[:, :]),
                     mybir.ImmediateValue(dtype=F32, value=0.0),
                     mybir.ImmediateValue(dtype=F32, value=1.0),
                     mybir.ImmediateValue(dtype=F32, value=0.0)],
                outs=[eng.lower_ap(warm[:, :])],
            ))

            # input DMAs; x ordered so K-half0/Q-chunk01 operands land first
            x_mm = pp.tile([128, 2 * N], sdt)
            w_mm = pp.tile([128, 3 * 256 + 512], sdt)
            for i, dsrc in enumerate((wq_in, wk_in, wv_in)):
                nc.gpsimd.dma_start(w_mm[:, i * 256:(i + 1) * 256], dsrc[:, :])
            nc.gpsimd.dma_start(w_mm[:, 768:1280], wp_in[:, :])
            nc.sync.dma_start(x_mm[:, 0:512], x_in[:, 0:512])
            nc.sync.dma_start(x_mm[:, N:N + 512], x_in[:, N:N + 512])
            nc.sync.dma_start(x_mm[:, 512:1024], x_in[:, 512:1024])
            nc.sync.dma_start(x_mm[:, N + 512:N + 1024], x_in[:, N + 512:N + 1024])
            nc.sync.dma_start(x_mm[:, 1024:N], x_in[:, 1024:N])
            nc.sync.dma_start(x_mm[:, N + 1024:2 * N], x_in[:, N + 1024:2 * N])
            wq_sb = w_mm[:, 0:256]
            wk_sb = w_mm[:, 256:512]
            wv_sb = w_mm[:, 512:768]
            wp_sb = w_mm[:, 768:1280]   # [wp1 | wp2] spread stationaries

            q_sb = pp.tile([128, N], sdt)
            k_sb = pp.tile([128, N], sdt)

            # V^T with ones column: per (head, mtile) a [128, 33] block.
            ones_f32 = pp.tile([128, 1], F32)
            nc.vector.memset(ones_f32[:, :], 1.0)
            vaug = pp.tile([128, HPC * MTILES * 33], sdt)
            nc.vector.tensor_copy(
                vaug.rearrange("p (b c) -> p b c", c=33)[:, :, 32:33],
                ones_f32[:, 0:1].to_broadcast([128, HPC * MTILES, 1]),
            )
            vaug_v = vaug.rearrange("p (h t c) -> p h t c", h=HPC, t=MTILES)

            # ones stationary for the K=1 recip-broadcast matmuls
            ones32 = pp.tile([128, 32], sdt)
            nc.vector.memset(ones32[:, :], 1.0)

            # AV accumulators keep partition bands 33-63 / 97-127 unwritten;
            # zero them once so the normalize mul stays finite forever.
            acc_init = []
            for tag in ("acc_ab", "acc_cd"):
                t = stp.tile([128, NCHUNK], F32, tag=tag, name=f"init_{tag}")
                nc.vector.memset(t[:, :], 0.0)
                acc_init.append(t)

            # ---- projection emitters (into the aux PSUM tile) --------------
            def emit_qk_half(dst, wsb, half):
                qp = stp.tile([128, 1024], F32, tag="st", bufs=3, name="qp")
                for s in range(2):
                    col0 = half * 1024 + s * 512
                    for cc in range(2):
                        _lbl(nc.tensor.matmul(
                            qp[:, s * 512:(s + 1) * 512],
                            wsb[:, cc * 128:(cc + 1) * 128],
                            x_mm[:, cc * N + col0: cc * N + col0 + 512],
                            start=(cc == 0), stop=(cc == 1),
                        ), f"QKPROJ h{half} s{s}")
                nc.vector.tensor_copy(dst[:, half * 1024:half * 1024 + 512],
                                      qp[:, 0:512])
                nc.vector.tensor_copy(dst[:, half * 1024 + 512:(half + 1) * 1024],
                                      qp[:, 512:1024])

            vhalf_ps = {}

            def emit_v_quarter(q):
                # 4 m-tiles of V^T; two quarters share one aux tile
                g, sub = q // 2, q % 2
                if sub == 0:
                    vhalf_ps[g] = stp.tile([128, 1024], F32, tag="st", bufs=3, name="vp")
                vp = vhalf_ps[g]
                for mtl in range(4):
                    vmt = q * 4 + mtl
                    for cc in range(2):
                        _lbl(nc.tensor.matmul(
                            vp[:, (sub * 4 + mtl) * 128:(sub * 4 + mtl + 1) * 128],
                            x_mm[:, cc * N + vmt * 128: cc * N + (vmt + 1) * 128],
                            wv_sb[:, cc * 128:(cc + 1) * 128],
                            start=(cc == 0), stop=(cc == 1),
                        ), f"VPROJ q{q} mtl{mtl}")
                if sub == 1:
                    nc.vector.tensor_copy(
                        vaug_v[:, :, g * 8:(g + 1) * 8, 0:32],
                        vp.rearrange("p (t h d) -> p h t d", t=8, h=HPC),
                    )

            def emit_exp(out_ap, in_ap, label="EXP"):
                ins = [eng.lower_ap(in_ap),
                       mybir.ImmediateValue(dtype=F32, value=0.0),
                       mybir.ImmediateValue(dtype=F32, value=1.0 / SCALE),
                       mybir.ImmediateValue(dtype=F32, value=0.0)]
                _lbl(eng.add_instruction(mybir.InstActivation(
                    name=nc.get_next_instruction_name(),
                    func=mybir.ActivationFunctionType.Exp,
                    ins=ins, outs=[eng.lower_ap(out_ap)],
                )), label)

            from concourse.alu_op_type import AluOpType

            def emit_dve_exp(ex_slice, st_tile):
                u = wk_pool.tile([128, 1024], sdt, tag="dveu", name="u")
                nc.vector.tensor_scalar_mul(
                    u[:, :], st_tile[:, :], 1.0 / (SCALE * 8.0))
                pa = wk_pool.tile([128, 1024], sdt, tag="dvepa", name="pa")
                pb = wk_pool.tile([128, 1024], sdt, tag="dvepb", name="pb")
                nc.vector.tensor_scalar_mul(pa[:, :], u[:, :], EXP_A)
                cur, nxt = pa, pb
                for coef in (EXP_B, EXP_C, EXP_D):
                    nc.vector.scalar_tensor_tensor(
                        nxt[:, :], cur[:, :], coef, u[:, :],
                        AluOpType.add, AluOpType.mult)
                    cur, nxt = nxt, cur
                for _ in range(3):
                    nc.vector.scalar_tensor_tensor(
                        nxt[:, :], cur[:, :], 2.0, cur[:, :],
                        AluOpType.add, AluOpType.mult)
                    cur, nxt = nxt, cur
                nc.vector.tensor_scalar_add(ex_slice, cur[:, :], 1.0)

            # ---- deferred normalize + projection ---------------------------
            # state for chunk j, filled during chunk j, consumed in chunk j+1
            norm_state = {}

            def emit_extract(j, tail=False):
                st = norm_state[j]
                den_sb = wk_pool.tile([128, NCHUNK], F32, tag="den", name="den_sb")
                st["den_sb"] = den_sb
                for i, acc in enumerate(st["accs"]):
                    for s in range(2):
                        r = 32 * (2 * i + s)
                        # at the tail ScalarE is idle: split the extraction
                        if tail and i == 0:
                            nc.scalar.copy(
                                den_sb[r:r + 1, :],
                                acc[s * 64 + 32:s * 64 + 33, :],
                            )
                        else:
                            nc.vector.tensor_copy(
                                den_sb[r:r + 1, :],
                                acc[s * 64 + 32:s * 64 + 33, :],
                            )

            def emit_recip(j):
                # 1/x = exp(-ln x) on ScalarE: Ln and Exp share one ACT table
                # set, and doing this on ScalarE keeps the DVE (and its drain)
                # off the muls' critical path.
                st = norm_state[j]
                den_sb = st["den_sb"]
                r_sb = wk_pool.tile([128, NCHUNK], sdt, tag="rsb", name="r_sb")
                ln_t = wk_pool.tile([128, NCHUNK], F32, tag="lnt", name="ln_t")
                nc.scalar.activation(
                    ln_t[:, :], den_sb[:, :], mybir.ActivationFunctionType.Ln)
                nc.scalar.activation(
                    r_sb[:, :], ln_t[:, :], mybir.ActivationFunctionType.Exp,
                    scale=-1.0)
                st["r_sb"] = r_sb
                if DEBUG and j == 0:
                    nc.sync.dma_start(dbg["dbg_den"][:, :], den_sb[:, :])
                    nc.sync.dma_start(dbg["dbg_r"][:, :], r_sb[:, :])

            def emit_bcast(j, tail=False):
                st = norm_state[j]
                r_sb = st["r_sb"]
                ps = stp.tile([128, 1024], F32, tag="st", bufs=3, name="psbc")
                for i in range(2):
                    for sub in range(2):
                        row = 32 * (2 * i + sub)
                        nc.tensor.matmul(
                            ps[64 * sub:64 * sub + 32, i * 512:(i + 1) * 512],
                            ones32[row:row + 1, :],
                            r_sb[row:row + 1, :],
                            tile_position=(row, 64 * sub),
                        )
                bc = wk_pool.tile([128, 1024], F32, tag="bc", name="bc")
                st["bc"] = bc
                nc.vector.tensor_copy(bc[:, :], ps[:, :])
                if DEBUG and j == 0:
                    nc.sync.dma_start(dbg["dbg_bc"][:, :], bc[:, :])

            def emit_norm(j):
                st = norm_state[j]
                bc = st["bc"]
                attn = wk_pool.tile([128, 1024], sdt, tag="attn", name="attn")
                st["attn"] = attn
                for i, acc in enumerate(st["accs"]):
                    nc.vector.tensor_mul(
                        attn[:, i * 512:(i + 1) * 512], acc[:, :],
                        bc[:, i * 512:(i + 1) * 512],
                    )
                if DEBUG and j == 0:
                    nc.sync.dma_start(dbg["dbg_attn"][:, :], attn[:, :])

            def emit_proj(j, tail=False):
                st = norm_state.pop(j)
                attn = st["attn"]
                n0 = j * NCHUNK
                yp = stp.tile([128, 1024], F32, tag="st", bufs=3, name="yp")
                for oh in range(2):
                    for i in range(2):
                        _lbl(nc.tensor.matmul(
                            yp[:, oh * 512:oh * 512 + NCHUNK],
                            wp_sb[:, i * 256 + oh * 128: i * 256 + (oh + 1) * 128],
                            attn[:, i * 512:(i + 1) * 512],
                            start=(i == 0), stop=(i == 1),
                        ), f"PROJ j{j} oh{oh} i{i}")
                for oh in range(2):
                    y_sb = wk_pool.tile([128, NCHUNK], sdt, tag="ysb", name="y_sb")
                    if tail and oh == 0:
                        nc.scalar.copy(y_sb[:, :], yp[:, oh * 512:(oh + 1) * 512])
                    else:
                        nc.vector.tensor_copy(y_sb[:, :], yp[:, oh * 512:(oh + 1) * 512])
                    nc.sync.dma_start(
                        y_out[oh * 128:(oh + 1) * 128, n0:n0 + NCHUNK],
                        y_sb[:, :],
                    )

            # ---- up-front projections (rest interleave into chunk 0) -------
            # K-half0 / Q-chunk01 interleaved at sub-512 granularity so the
            # first S^T (needs k[:, 0:128] + q[:, 0:512]) fires after only
            # 4 matmuls + 2 copies instead of the full 8+4 chain.
            kp0 = stp.tile([128, 1024], F32, tag="st", bufs=3, name="kp0")
            qp0 = stp.tile([128, 1024], F32, tag="st", bufs=3, name="qp0")
            for sub in range(2):
                for dst_ps, wsb in ((kp0, wk_sb), (qp0, wq_sb)):
                    for cc in range(2):
                        _lbl(nc.tensor.matmul(
                            dst_ps[:, sub * 512:(sub + 1) * 512],
                            wsb[:, cc * 128:(cc + 1) * 128],
                            x_mm[:, cc * N + sub * 512: cc * N + (sub + 1) * 512],
                            start=(cc == 0), stop=(cc == 1),
                        ), f"QK0 s{sub}")
                nc.vector.tensor_copy(
                    k_sb[:, sub * 512:(sub + 1) * 512],
                    kp0[:, sub * 512:(sub + 1) * 512])
                nc.vector.tensor_copy(
                    q_sb[:, sub * 512:(sub + 1) * 512],
                    qp0[:, sub * 512:(sub + 1) * 512])

            # per-slot extra work: (j, mt) -> list of callables
            slot_work = {}
            for j in range(NJ):
                if j == 0:
                    slot_work[(0, 0)] = [lambda: emit_v_quarter(0)]
                    slot_work[(0, 1)] = [lambda: emit_v_quarter(1)]
                    slot_work[(0, 3)] = [lambda: emit_qk_half(k_sb, wk_sb, 1)]
                    slot_work[(0, 5)] = [lambda: emit_v_quarter(2)]
                    slot_work[(0, 6)] = [lambda: emit_v_quarter(3)]
                    slot_work[(0, 9)] = [lambda: emit_qk_half(q_sb, wq_sb, 1)]
                else:
                    jm = j - 1
                    slot_work[(j, 1)] = [lambda jm=jm: emit_extract(jm)]
                    slot_work[(j, 3)] = [lambda jm=jm: emit_recip(jm)]
                    slot_work[(j, 4)] = [lambda jm=jm: emit_bcast(jm)]
                    slot_work[(j, 5)] = [lambda jm=jm: emit_norm(jm)]
                    slot_work[(j, 6)] = [lambda jm=jm: emit_proj(jm)]

            # ---- main attention loop ---------------------------------------
            # leftover AV closures carried into the next chunk's first slots
            carry_av = []
            for j in range(NJ):
                n0 = j * NCHUNK
                acc_ab = stp.tile([128, NCHUNK], F32, tag="acc_ab", name="acc_ab")
                acc_cd = stp.tile([128, NCHUNK], F32, tag="acc_cd", name="acc_cd")
                accs = [acc_ab, acc_cd]
                norm_state[j] = {"accs": accs}
                ex_tiles = {}
                pending_av = []
                deferred_av = {}

                def emit_av(mt, accs=accs, ex_tiles=ex_tiles):
                    ex_mt = ex_tiles.pop(mt)
                    for pair in range(2):
                        acc = accs[pair]
                        for sub in range(2):
                            h = 2 * pair + sub
                            _lbl(nc.tensor.matmul(
                                acc[sub * 64:sub * 64 + 33, :],
                                vaug_v[:, h, mt, :],
                                ex_mt[:, h * 512:(h + 1) * 512],
                                start=(mt == 0), stop=(mt == MTILES - 1),
                                tile_position=(0, sub * 64),
                            ), f"AV mt{mt} h{h}")

                for mt in range(MTILES):
                    # S^T first: all four heads' matmuls go back-to-back so
                    # the PE runs them as 4 concurrent row-tiles. Emitted at
                    # boosted priority so the scheduler never parks the exp
                    # feed behind AV / normalize work.
                    ex = xp.tile([128, 2048], sdt, name="ex")
                    ex_tiles[mt] = ex
                    with tc.high_priority(offset=PRIO_OFFSET):
                        st_1 = stp.tile([128, 1024], F32, tag="st", bufs=3, name="st_1")
                        st_2 = stp.tile([128, 1024], F32, tag="st", bufs=3, name="st_2")
                        for h in range(4):
                            dst = st_1 if h < 2 else st_2
                            _lbl(nc.tensor.matmul(
                                dst[:, (h % 2) * 512:(h % 2 + 1) * 512],
                                k_sb[h * 32:(h + 1) * 32, mt * 128:(mt + 1) * 128],
                                q_sb[h * 32:(h + 1) * 32, n0:n0 + NCHUNK],
                                tile_position=(32 * h, 0),
                            ), f"ST j{j} mt{mt} h{h}")
                        if mt in OFFLOAD_MTS:
                            emit_dve_exp(ex[:, 0:1024], st_1)
                        else:
                            emit_exp(ex[:, 0:1024], st_1[:, :], f"EXP j{j} mt{mt} a")
                        emit_exp(ex[:, 1024:2048], st_2[:, :], f"EXP j{j} mt{mt} b")
                    if mt in OFFLOAD_MTS:
                        deferred_av[OFFLOAD_MTS[mt]] = mt
                    else:
                        pending_av.append(mt)

                    # previous chunk's leftover AV matmuls at slot 0
                    if mt == 0:
                        while carry_av:
                            carry_av.pop(0)()
                    if mt in deferred_av:
                        emit_av(deferred_av.pop(mt))
                    # AV drain: up to 2 one-slot-lagged m-tiles per slot
                    if mt >= FIRST_AV_SLOT:
                        for _ in range(2):
                            if pending_av and pending_av[0] <= mt - 1:
                                emit_av(pending_av.pop(0))

                    if DEBUG and j == 0 and mt == 0:
                        nc.sync.dma_start(dbg["dbg_ex"][:, :], ex[:, :])
                    if DEBUG and j == 0 and mt == 15:
                        nc.sync.dma_start(dbg["dbg_q"][:, :], q_sb[:, :])
                        nc.sync.dma_start(dbg["dbg_k"][:, :], k_sb[:, :])
                        nc.sync.dma_start(dbg["dbg_v"][:, :], vaug[:, :])

                    for fn in slot_work.get((j, mt), ()):
                        fn()

                if j < NJ - 1:
                    for mt_left in pending_av:
                        carry_av.append(lambda m=mt_left, e=emit_av: e(m))
                else:
                    while pending_av:
                        emit_av(pending_av.pop(0))

            # tail: last chunk's normalize + projection
            emit_extract(NJ - 1, tail=True)
            emit_recip(NJ - 1)
            emit_bcast(NJ - 1, tail=True)
            emit_norm(NJ - 1)
            emit_proj(NJ - 1, tail=True)

    _split_sync_waits(nc)
    return nc


_CACHE = {}


def _get_program():
    if "nc" not in _CACHE:
        _CACHE["nc"] = build_program()
    return _CACHE["nc"]


def _core_inputs(x, w_qkv, w_proj, core):
    b, g = core // 2, core % 2
    r0 = g * 128
    wq = w_qkv[r0:r0 + 128, :].T            # [256 c, 128 (h,d)]
    wk = w_qkv[256 + r0:256 + r0 + 128, :].T
    wv = w_qkv[512 + r0:512 + r0 + 128, :].T
    wpj = w_proj[:, r0:r0 + 128].T          # [128 c_local, 256 o]

    hdt = mybir.dt.np(MM_DT)

    def chunk_c(a):  # [256, m] -> [128, 2*m] with c split across 2 free-chunks
        m = a.shape[1]
        return np.ascontiguousarray(
            a.reshape(2, 128, m).transpose(1, 0, 2).reshape(128, 2 * m)
        ).astype(hdt)

    # spread + zero-pad the projection stationaries: head-pair i uses rows
    # {0-31: h_even dims, 64-95: h_odd dims}, other bands killed by zeros
    wp = np.zeros((128, 512), dtype=np.float32)
    for i in range(2):                      # head pair
        for s in range(2):                  # even/odd head in pair
            wp[s * 64:s * 64 + 32, i * 256:(i + 1) * 256] = \
                wpj[(2 * i + s) * 32:(2 * i + s + 1) * 32, :]

    return {
        "x_in": chunk_c(x[b]),
        "wq_in": chunk_c(wq),
        "wk_in": chunk_c(wk),
        "wv_in": chunk_c(wv),
        "wp_in": wp.astype(hdt),
    }


def kernel(x, w_qkv, w_proj, n_heads=8, _trace=False):
    x = np.asarray(x, dtype=np.float32)
    w_qkv = np.asarray(w_qkv, dtype=np.float32)
    w_proj = np.asarray(w_proj, dtype=np.float32)
    assert int(n_heads) == H

    nc = _get_program()
    in_maps = [_core_inputs(x, w_qkv, w_proj, core) for core in range(NCORES)]
    res = bass_utils.run_bass_kernel_spmd(
        nc, in_maps, core_ids=list(range(NCORES)), trace=_trace
    )
    parts = [np.asarray(res.results[core]["y_out"], dtype=np.float32)
             for core in range(NCORES)]
    y = np.stack([parts[2 * b] + parts[2 * b + 1] for b in range(B)])
    if _trace:
        kernel.last_result = res
    return y.astype(np.float32)
